# revision 18
# baseline (speedup 1.0000x reference)
"""MoE top-1 routing kernel for Trainium2 (8 NeuronCores, expert-parallel).

Strategy:
  - Gate (x @ Wg.T + bg, argmax) is computed on host in float64. The min
    top-2 logit gap for this problem's data is ~1.2e-5, orders of magnitude
    above any fp32 backend's rounding noise (~1e-6), so the fp64 argmax
    matches the fp32 reference argmax exactly.
  - Tokens are grouped by expert on host (the "all-to-all dispatch");
    core e receives expert e's tokens (capacity-padded) plus expert e's
    weights, and runs the dense SwiGLU FFN for just those tokens.
  - Outputs are scattered back to token order on host (the "combine").
    With top-1 routing the combine weight is exactly 1.0.

Device kernel (per core), all matmuls on the PE array in bf16 (full
1 cycle/row rate at any moving width; rel-err ~4e-3 end to end):
  h1^T = W1 x^T   (contract D, f on partitions)
  h2^T = W2 x^T
  g^T  = silu(h1^T) * h2^T
  y^T  = W3 g^T    (contract F, d on partitions)
All tensors are staged transposed (feature-major) so the PE contraction
dim always sits on partitions; the host does the transposes.

Schedule notes (cost-model driven):
  - HWDGE serializes every DMA at ~625ns regardless of size, so loads are
    merged into k-pair / whole-tensor DMAs and stores into one merged DMA
    per chunk.
  - The PE clock ramps (1.54 -> 0.83 -> 0.417 ns/row) and reaches full
    speed only after 3us of continuous busy; dummy warmup matmuls burn the
    ramp during the DMA preload window.
  - The last chunk is small (128 tokens) and its d-tile stores fan out
    across the SP/Act/DVE issue queues so the end-of-kernel store+drain
    tail shrinks.
"""

import sys
from contextlib import ExitStack

if "/opt/trn_rl_repo" not in sys.path:
    sys.path.insert(0, "/opt/trn_rl_repo")

import numpy as np

P = 128
D = 768          # model dim
E = 8            # experts == cores
F = 469          # ffn hidden
FP = 512         # F padded to a multiple of 128
KT = D // P      # 6 k-tiles over D
MT = FP // P     # 4 f-tiles over padded F
DT = D // P      # 6 out-tiles over D
KP = KT // 2     # 3 k-pairs (DMA granularity for weight/x0 preload)
MIN_C = 128      # capacity floor; actual C adapts to max expert load
CHUNK = 512      # moving-operand free dim per matmul (== one PSUM bank f32)
TAIL = 128       # final chunk size (small => short store tail)

MM_MODE = "bfloat16"   # matmul input precision on device
WARMUP_MMS = 15        # dummy matmuls to pre-warm the PE clock during preload
WARM_COLS = 256

# pool buffer counts
BUFS = {"x": 1, "g": 2, "s": 4, "o": 2, "ps": 8}

_cache = {}


def _np_in_dtype():
    if MM_MODE == "bfloat16":
        import ml_dtypes

        return np.dtype(ml_dtypes.bfloat16)
    return np.dtype(np.float32)


def _chunk_sizes(C):
    """[~512]*n + [TAIL]; all even, sum == C."""
    assert C % 2 == 0, C
    if C <= CHUNK:
        return [C]
    body = C - TAIL
    nb = -(-body // CHUNK)
    u, rem = divmod(body // 2, nb)
    sizes = [2 * (u + (1 if i < rem else 0)) for i in range(nb)]
    sizes.append(TAIL)
    assert sum(sizes) == C
    return sizes


def _build(C):
    """Build + compile the per-core Tile kernel for capacity C tokens."""
    import concourse.bacc as bacc
    import concourse.tile as tile
    from concourse import mybir

    f32 = mybir.dt.float32
    in_dt = {
        "bfloat16": mybir.dt.bfloat16,
        "float32r": mybir.dt.float32r,
        "float32": mybir.dt.float32,
    }[MM_MODE]

    nc = bacc.Bacc("TRN2", target_bir_lowering=False, debug=False, num_devices=E)

    xt = nc.dram_tensor("xt", [KT, P, C], in_dt, kind="ExternalInput").ap()
    w1t = nc.dram_tensor("w1t", [KT, P, FP], in_dt, kind="ExternalInput").ap()
    w2t = nc.dram_tensor("w2t", [KT, P, FP], in_dt, kind="ExternalInput").ap()
    w3t = nc.dram_tensor("w3t", [MT, P, D], in_dt, kind="ExternalInput").ap()
    yt = nc.dram_tensor("yt", [DT, P, C], f32, kind="ExternalOutput").ap()

    sizes = _chunk_sizes(C)
    chunks = []
    off = 0
    for nn in sizes:
        chunks.append((off, nn))
        off += nn
    nn0 = chunks[0][1]
    silu = mybir.ActivationFunctionType.Silu

    with tile.TileContext(nc) as tc, ExitStack() as ctx:
        wpool = ctx.enter_context(tc.tile_pool(name="w", bufs=1))
        xpool = ctx.enter_context(tc.tile_pool(name="x", bufs=BUFS["x"]))
        gpool = ctx.enter_context(tc.tile_pool(name="g", bufs=BUFS["g"]))
        spool = ctx.enter_context(tc.tile_pool(name="s", bufs=BUFS["s"]))
        opool = ctx.enter_context(tc.tile_pool(name="o", bufs=BUFS["o"]))
        pspool = ctx.enter_context(
            tc.tile_pool(name="ps", bufs=BUFS["ps"], space="PSUM")
        )

        # weights/x0 at k-pair granularity: one HWDGE slot (~625ns) serves
        # two k-tiles, keeping DMA delivery ahead of PE consumption
        w1p = [
            wpool.tile([P, 2, FP], in_dt, tag=f"w1_{kp}", name=f"w1_{kp}")
            for kp in range(KP)
        ]
        w2p = [
            wpool.tile([P, 2, FP], in_dt, tag=f"w2_{kp}", name=f"w2_{kp}")
            for kp in range(KP)
        ]
        w3_sb = wpool.tile([P, MT, D], in_dt, tag="w3", name="w3_sb")
        x0p = [
            xpool.tile([P, 2, nn0], in_dt, tag=f"x0_{kp}", name=f"x0_{kp}")
            for kp in range(KP)
        ]
        xn_t = [
            xpool.tile([P, KT, nn], in_dt, tag=f"xc_{c}", name=f"xc_{c}")
            for c, (_, nn) in enumerate(chunks)
            if c > 0
        ]

        def w1s(k, m):
            return w1p[k // 2][:, k % 2, m * P : (m + 1) * P]

        def w2s(k, m):
            return w2p[k // 2][:, k % 2, m * P : (m + 1) * P]

        def w3s(m, d):
            return w3_sb[:, m, d * P : (d + 1) * P]

        # dummy matmuls on a zeroed tile bridge the DMA-preload window so the
        # PE p-state ramp (full speed after 3us continuous busy) is already
        # paid when the first real matmul issues; outputs are never used
        warm = wpool.tile([P, WARM_COLS], in_dt, tag="warm", name="warm")
        nc.gpsimd.memset(warm[:], 0.0)
        wps = pspool.tile([P, WARM_COLS], f32, tag="ps", name="wps")
        for _ in range(WARMUP_MMS):
            nc.tensor.matmul(wps[:], warm[:, :P], warm[:], start=True, stop=True)
        wsink = wpool.tile([P, WARM_COLS], f32, tag="wsink", name="wsink")
        nc.scalar.copy(wsink[:], wps[:])  # consume so the PSUM slot frees

        # ---- preload: all loads on the SP queue, in consumption order ----
        for kp in range(KP):
            if kp == 0:
                # split the first w1 pair in f-halves: the m01 half's short
                # transfer lets x0's transfer (the first-matmul gate) start
                # ~0.4us earlier on the serial DMA engine pipe
                nc.sync.dma_start(
                    w1p[0][:, :, : FP // 2],
                    w1t[0:2, :, : FP // 2].rearrange("k p f -> p k f"),
                )
            else:
                nc.sync.dma_start(
                    w1p[kp][:], w1t[2 * kp : 2 * kp + 2].rearrange("k p f -> p k f")
                )
            nc.sync.dma_start(
                x0p[kp][:],
                xt[2 * kp : 2 * kp + 2, :, 0:nn0].rearrange("k p n -> p k n"),
            )
            if kp == 0:
                nc.sync.dma_start(
                    w1p[0][:, :, FP // 2 :],
                    w1t[0:2, :, FP // 2 :].rearrange("k p f -> p k f"),
                )
        for kp in range(KP):
            nc.sync.dma_start(
                w2p[kp][:], w2t[2 * kp : 2 * kp + 2].rearrange("k p f -> p k f")
            )
        nc.sync.dma_start(w3_sb[:], w3t.rearrange("m p d -> p m d"))
        for c, (n0, nn) in enumerate(chunks):
            if c == 0:
                continue
            nc.sync.dma_start(
                xn_t[c - 1][:], xt[:, :, n0 : n0 + nn].rearrange("k p n -> p k n")
            )

        # per-chunk emission helpers; ps1 pair-0 of chunk c+1 is hoisted
        # before stage B of chunk c so the PE never idles at chunk
        # boundaries waiting for the silu->mul chain to produce g
        def xs(c, k):
            if c == 0:
                return x0p[k // 2][:, k % 2, :]
            return xn_t[c - 1][:, k, :]

        def emit_ps1_pair(c, ms):
            _, nn = chunks[c]
            ps1 = {
                m: pspool.tile([P, nn], f32, tag="ps", name=f"ps1_{c}_{m}")
                for m in ms
            }
            for k in range(KT):
                for m in ms:
                    nc.tensor.matmul(
                        ps1[m][:], w1s(k, m), xs(c, k),
                        start=(k == 0), stop=(k == KT - 1),
                    )
            return ps1

        def emit_ps2_pair_and_g(c, ms, ps1, gs):
            _, nn = chunks[c]
            ps2 = {
                m: pspool.tile([P, nn], f32, tag="ps", name=f"ps2_{c}_{m}")
                for m in ms
            }
            for k in range(KT):
                for m in ms:
                    nc.tensor.matmul(
                        ps2[m][:], w2s(k, m), xs(c, k),
                        start=(k == 0), stop=(k == KT - 1),
                    )
            for m in ms:
                sil = spool.tile([P, nn], f32, tag="sil", name="sil")
                nc.scalar.activation(sil[:], ps1[m][:], silu)
                g = gpool.tile([P, nn], in_dt, tag=f"g{m}", name=f"g{m}")
                nc.vector.tensor_mul(g[:], sil[:], ps2[m][:])
                gs.append(g)

        def emit_stage_a0(gs):
            # chunk 0: fused W1+W2 k-loop over all 4 f-tiles so the PE
            # consumes one (w1,x0,w2) k-pair DMA triplet per 16 matmuls,
            # staying just behind the HWDGE delivery rate during preload
            nn = chunks[0][1]
            ps1 = {
                m: pspool.tile([P, nn], f32, tag="ps", name=f"ps1_0_{m}")
                for m in range(MT)
            }
            ps2 = {
                m: pspool.tile([P, nn], f32, tag="ps", name=f"ps2_0_{m}")
                for m in range(MT)
            }
            for k in range(KT):
                for m in range(MT):
                    nc.tensor.matmul(
                        ps1[m][:], w1s(k, m), xs(0, k),
                        start=(k == 0), stop=(k == KT - 1),
                    )
                for m in range(MT):
                    nc.tensor.matmul(
                        ps2[m][:], w2s(k, m), xs(0, k),
                        start=(k == 0), stop=(k == KT - 1),
                    )
            for m in range(MT):
                sil = spool.tile([P, nn], f32, tag="sil", name="sil")
                nc.scalar.activation(sil[:], ps1[m][:], silu)
                g = gpool.tile([P, nn], in_dt, tag=f"g{m}", name=f"g{m}")
                nc.vector.tensor_mul(g[:], sil[:], ps2[m][:])
                gs.append(g)

        def emit_stage_b(c, gs):
            n0, nn = chunks[c]
            last = c == len(chunks) - 1
            ot = opool.tile([P, DT, nn], f32, tag="ot", name="ot")
            # tail d-pair stores fan across SP/Act/Pool so the final chain
            # isn't serialized on one SEQ or on the shared HWDGE device
            # (Pool goes via SWDGE, bypassing HWDGE)
            if last:
                # tail chunk: an m0 round across all d first bridges the
                # trailing g2/g3 silu->mul latency; stores grouped so the
                # very last store (gated by d5's copy) is the smallest
                # possible transfer on its own free queue (Act), with d0-2
                # on SP/HWDGE and d3-4 on Pool/SWDGE ahead of it
                psos = [
                    pspool.tile([P, nn], f32, tag="ps", name=f"pso{d}")
                    for d in range(DT)
                ]
                for d in range(DT):
                    nc.tensor.matmul(
                        psos[d][:], w3s(0, d), gs[0][:], start=True, stop=False
                    )
                for d in range(DT):
                    for m in range(1, MT):
                        nc.tensor.matmul(
                            psos[d][:], w3s(m, d), gs[m][:],
                            start=False, stop=(m == MT - 1),
                        )
                    if d % 2 == 1:
                        nc.scalar.copy(ot[:, d, :], psos[d][:])
                    else:
                        nc.vector.tensor_copy(ot[:, d, :], psos[d][:])
                    if d % 2 == 1:
                        tail_eng = [nc.sync, nc.gpsimd, nc.scalar]
                        tail_eng[d // 2].dma_start(
                            yt[d - 1 : d + 1, :, n0 : n0 + nn].rearrange(
                                "d p n -> p d n"
                            ),
                            ot[:, d - 1 : d + 1, :],
                        )
                return
            for d in range(DT):
                pso = pspool.tile([P, nn], f32, tag="ps", name="pso")
                for m in range(MT):
                    nc.tensor.matmul(
                        pso[:], w3s(m, d), gs[m][:],
                        start=(m == 0), stop=(m == MT - 1),
                    )
                if d % 2 == 1:
                    nc.scalar.copy(ot[:, d, :], pso[:])
                else:
                    nc.vector.tensor_copy(ot[:, d, :], pso[:])
                if d % 2 == 1:
                    # store each d-pair as soon as both copies land. Thirds
                    # (not one merged store) so no single transfer hogs the
                    # serial DMA_ENGINES device into the next chunk's window.
                    # All on SP: idle after preload, and wait order matches
                    # time order there -- stores must NOT sit on Act/DVE
                    # queues where in-order SEQs would block compute
                    # dispatch behind the store's copy-waits.
                    nc.sync.dma_start(
                        yt[d - 1 : d + 1, :, n0 : n0 + nn].rearrange(
                            "d p n -> p d n"
                        ),
                        ot[:, d - 1 : d + 1, :],
                    )

        NCH = len(chunks)
        gs_all = [[] for _ in range(NCH)]
        hoisted = [None] * NCH
        emit_stage_a0(gs_all[0])
        for c in range(NCH):
            if c + 1 < NCH:
                # hoist: next chunk's ps1 pair-0 k-loop before this stage B
                hoisted[c + 1] = emit_ps1_pair(c + 1, (0, 1))
            emit_stage_b(c, gs_all[c])
            if c + 1 < NCH:
                emit_ps2_pair_and_g(c + 1, (0, 1), hoisted[c + 1], gs_all[c + 1])
                ps1b = emit_ps1_pair(c + 1, (2, 3))
                emit_ps2_pair_and_g(c + 1, (2, 3), ps1b, gs_all[c + 1])

    nc.compile()
    return nc


LAST_RESULTS = None  # BassKernelResults of the most recent run (for test harness)


def kernel(x, Wg, bg, W1, W2, W3):
    global LAST_RESULTS
    from concourse.bass_utils import run_bass_kernel_spmd

    x = np.asarray(x)
    Wg, bg = np.asarray(Wg), np.asarray(bg)
    W1, W2, W3 = np.asarray(W1), np.asarray(W2), np.asarray(W3)
    B, S, d = x.shape
    T = B * S
    assert d == D and Wg.shape == (E, D)

    xf = np.ascontiguousarray(x.reshape(T, D))

    # ---- host gate + top-1 routing (fp64: exact vs any fp32 backend) ----
    gate = xf.astype(np.float64) @ Wg.astype(np.float64).T + bg.astype(np.float64)
    eid = np.argmax(gate, axis=1)
    counts = np.bincount(eid, minlength=E)
    order = np.argsort(eid, kind="stable")
    offs = np.concatenate(([0], np.cumsum(counts)))

    C = max(MIN_C, 2 * int(-(-counts.max() // 2)))
    key = (C, MM_MODE)
    if key not in _cache:
        _cache[key] = _build(C)
    nc = _cache[key]

    in_dt = _np_in_dtype()

    # ---- build per-core inputs (dispatch) ----
    in_maps = []
    tok_lists = []
    for e in range(E):
        toks = order[offs[e] : offs[e + 1]]
        tok_lists.append(toks)
        ce = len(toks)
        xeT = np.zeros((D, C), dtype=in_dt)
        if ce:
            xeT[:, :ce] = xf[toks].T.astype(in_dt)
        w1 = np.zeros((D, FP), dtype=in_dt)
        w1[:, :F] = W1[e].T.astype(in_dt)
        w2 = np.zeros((D, FP), dtype=in_dt)
        w2[:, :F] = W2[e].T.astype(in_dt)
        w3 = np.zeros((FP, D), dtype=in_dt)
        w3[:F, :] = W3[e].T.astype(in_dt)
        in_maps.append(
            {
                "xt": np.ascontiguousarray(xeT.reshape(KT, P, C)),
                "w1t": np.ascontiguousarray(w1.reshape(KT, P, FP)),
                "w2t": np.ascontiguousarray(w2.reshape(KT, P, FP)),
                "w3t": np.ascontiguousarray(w3.reshape(MT, P, D)),
            }
        )

    res = run_bass_kernel_spmd(nc, in_maps, list(range(E)))
    LAST_RESULTS = res

    # ---- combine: scatter outputs back to token order ----
    y = np.empty((T, D), dtype=np.float32)
    for e in range(E):
        toks = tok_lists[e]
        if len(toks):
            yte = res.results[e]["yt"].reshape(D, C)
            y[toks] = yte[:, : len(toks)].T
    return y.reshape(B, S, d)


# revision 19
# speedup vs baseline: 1.0010x; 1.0010x over previous
"""MoE top-1 routing kernel for Trainium2 (8 NeuronCores, expert-parallel).

Strategy:
  - Gate (x @ Wg.T + bg, argmax) is computed on host in float64. The min
    top-2 logit gap for this problem's data is ~1.2e-5, orders of magnitude
    above any fp32 backend's rounding noise (~1e-6), so the fp64 argmax
    matches the fp32 reference argmax exactly.
  - Tokens are grouped by expert on host (the "all-to-all dispatch");
    core e receives expert e's tokens (capacity-padded) plus expert e's
    weights, and runs the dense SwiGLU FFN for just those tokens.
  - Outputs are scattered back to token order on host (the "combine").
    With top-1 routing the combine weight is exactly 1.0.

Device kernel (per core), all matmuls on the PE array in bf16 (full
1 cycle/row rate at any moving width; rel-err ~4e-3 end to end):
  h1^T = W1 x^T   (contract D, f on partitions)
  h2^T = W2 x^T
  g^T  = silu(h1^T) * h2^T
  y^T  = W3 g^T    (contract F, d on partitions)
All tensors are staged transposed (feature-major) so the PE contraction
dim always sits on partitions; the host does the transposes.

Schedule notes (cost-model driven):
  - HWDGE serializes every DMA at ~625ns regardless of size, so loads are
    merged into k-pair / whole-tensor DMAs and stores into one merged DMA
    per chunk.
  - The PE clock ramps (1.54 -> 0.83 -> 0.417 ns/row) and reaches full
    speed only after 3us of continuous busy; dummy warmup matmuls burn the
    ramp during the DMA preload window.
  - The last chunk is small (128 tokens) and its d-tile stores fan out
    across the SP/Act/DVE issue queues so the end-of-kernel store+drain
    tail shrinks.
"""

import sys
from contextlib import ExitStack

if "/opt/trn_rl_repo" not in sys.path:
    sys.path.insert(0, "/opt/trn_rl_repo")

import numpy as np

P = 128
D = 768          # model dim
E = 8            # experts == cores
F = 469          # ffn hidden
FP = 512         # F padded to a multiple of 128
KT = D // P      # 6 k-tiles over D
MT = FP // P     # 4 f-tiles over padded F
DT = D // P      # 6 out-tiles over D
KP = KT // 2     # 3 k-pairs (DMA granularity for weight/x0 preload)
MIN_C = 128      # capacity floor; actual C adapts to max expert load
CHUNK = 512      # moving-operand free dim per matmul (== one PSUM bank f32)
TAIL = 128       # final chunk size (small => short store tail)

MM_MODE = "bfloat16"   # matmul input precision on device
WARMUP_MMS = 15        # dummy matmuls to pre-warm the PE clock during preload
WARM_COLS = 256

# pool buffer counts
BUFS = {"x": 1, "g": 2, "s": 4, "o": 2, "ps": 8}

_cache = {}


def _np_in_dtype():
    if MM_MODE == "bfloat16":
        import ml_dtypes

        return np.dtype(ml_dtypes.bfloat16)
    return np.dtype(np.float32)


def _chunk_sizes(C):
    """[~512]*n + [TAIL]; all even, sum == C."""
    assert C % 2 == 0, C
    if C <= CHUNK:
        return [C]
    body = C - TAIL
    nb = -(-body // CHUNK)
    u, rem = divmod(body // 2, nb)
    sizes = [2 * (u + (1 if i < rem else 0)) for i in range(nb)]
    sizes.append(TAIL)
    assert sum(sizes) == C
    return sizes


def _build(C):
    """Build + compile the per-core Tile kernel for capacity C tokens."""
    import concourse.bacc as bacc
    import concourse.tile as tile
    from concourse import mybir

    f32 = mybir.dt.float32
    in_dt = {
        "bfloat16": mybir.dt.bfloat16,
        "float32r": mybir.dt.float32r,
        "float32": mybir.dt.float32,
    }[MM_MODE]

    nc = bacc.Bacc("TRN2", target_bir_lowering=False, debug=False, num_devices=E)

    xt = nc.dram_tensor("xt", [KT, P, C], in_dt, kind="ExternalInput").ap()
    w1t = nc.dram_tensor("w1t", [KT, P, FP], in_dt, kind="ExternalInput").ap()
    w2t = nc.dram_tensor("w2t", [KT, P, FP], in_dt, kind="ExternalInput").ap()
    w3t = nc.dram_tensor("w3t", [MT, P, D], in_dt, kind="ExternalInput").ap()
    yt = nc.dram_tensor("yt", [DT, P, C], f32, kind="ExternalOutput").ap()

    sizes = _chunk_sizes(C)
    chunks = []
    off = 0
    for nn in sizes:
        chunks.append((off, nn))
        off += nn
    nn0 = chunks[0][1]
    silu = mybir.ActivationFunctionType.Silu

    with tile.TileContext(nc) as tc, ExitStack() as ctx:
        wpool = ctx.enter_context(tc.tile_pool(name="w", bufs=1))
        xpool = ctx.enter_context(tc.tile_pool(name="x", bufs=BUFS["x"]))
        gpool = ctx.enter_context(tc.tile_pool(name="g", bufs=BUFS["g"]))
        spool = ctx.enter_context(tc.tile_pool(name="s", bufs=BUFS["s"]))
        opool = ctx.enter_context(tc.tile_pool(name="o", bufs=BUFS["o"]))
        pspool = ctx.enter_context(
            tc.tile_pool(name="ps", bufs=BUFS["ps"], space="PSUM")
        )

        # weights/x0 at k-pair granularity: one HWDGE slot (~625ns) serves
        # two k-tiles, keeping DMA delivery ahead of PE consumption
        w1p = [
            wpool.tile([P, 2, FP], in_dt, tag=f"w1_{kp}", name=f"w1_{kp}")
            for kp in range(KP)
        ]
        w2p = [
            wpool.tile([P, 2, FP], in_dt, tag=f"w2_{kp}", name=f"w2_{kp}")
            for kp in range(KP)
        ]
        w3_sb = wpool.tile([P, MT, D], in_dt, tag="w3", name="w3_sb")
        x0p = [
            xpool.tile([P, 2, nn0], in_dt, tag=f"x0_{kp}", name=f"x0_{kp}")
            for kp in range(KP)
        ]
        xn_t = [
            xpool.tile([P, KT, nn], in_dt, tag=f"xc_{c}", name=f"xc_{c}")
            for c, (_, nn) in enumerate(chunks)
            if c > 0
        ]

        def w1s(k, m):
            return w1p[k // 2][:, k % 2, m * P : (m + 1) * P]

        def w2s(k, m):
            return w2p[k // 2][:, k % 2, m * P : (m + 1) * P]

        def w3s(m, d):
            return w3_sb[:, m, d * P : (d + 1) * P]

        # dummy matmuls on a zeroed tile bridge the DMA-preload window so the
        # PE p-state ramp (full speed after 3us continuous busy) is already
        # paid when the first real matmul issues; outputs are never used
        warm = wpool.tile([P, WARM_COLS], in_dt, tag="warm", name="warm")
        nc.gpsimd.memset(warm[:], 0.0)
        wps = pspool.tile([P, WARM_COLS], f32, tag="ps", name="wps")
        for _ in range(WARMUP_MMS):
            nc.tensor.matmul(wps[:], warm[:, :P], warm[:], start=True, stop=True)
        wsink = wpool.tile([P, WARM_COLS], f32, tag="wsink", name="wsink")
        nc.scalar.copy(wsink[:], wps[:])  # consume so the PSUM slot frees

        # ---- preload: all loads on the SP queue, in consumption order ----
        for kp in range(KP):
            if kp == 0:
                # split the first w1 pair in f-halves: the m01 half's short
                # transfer lets x0's transfer (the first-matmul gate) start
                # ~0.4us earlier on the serial DMA engine pipe
                nc.sync.dma_start(
                    w1p[0][:, :, : FP // 2],
                    w1t[0:2, :, : FP // 2].rearrange("k p f -> p k f"),
                )
            else:
                nc.sync.dma_start(
                    w1p[kp][:], w1t[2 * kp : 2 * kp + 2].rearrange("k p f -> p k f")
                )
            nc.sync.dma_start(
                x0p[kp][:],
                xt[2 * kp : 2 * kp + 2, :, 0:nn0].rearrange("k p n -> p k n"),
            )
            if kp == 0:
                nc.sync.dma_start(
                    w1p[0][:, :, FP // 2 :],
                    w1t[0:2, :, FP // 2 :].rearrange("k p f -> p k f"),
                )
        for kp in range(KP):
            nc.sync.dma_start(
                w2p[kp][:], w2t[2 * kp : 2 * kp + 2].rearrange("k p f -> p k f")
            )
        nc.sync.dma_start(w3_sb[:], w3t.rearrange("m p d -> p m d"))
        for c, (n0, nn) in enumerate(chunks):
            if c == 0:
                continue
            nc.sync.dma_start(
                xn_t[c - 1][:], xt[:, :, n0 : n0 + nn].rearrange("k p n -> p k n")
            )

        # per-chunk emission helpers; ps1 pair-0 of chunk c+1 is hoisted
        # before stage B of chunk c so the PE never idles at chunk
        # boundaries waiting for the silu->mul chain to produce g
        def xs(c, k):
            if c == 0:
                return x0p[k // 2][:, k % 2, :]
            return xn_t[c - 1][:, k, :]

        def emit_ps1_pair(c, ms):
            _, nn = chunks[c]
            ps1 = {
                m: pspool.tile([P, nn], f32, tag="ps", name=f"ps1_{c}_{m}")
                for m in ms
            }
            for k in range(KT):
                for m in ms:
                    nc.tensor.matmul(
                        ps1[m][:], w1s(k, m), xs(c, k),
                        start=(k == 0), stop=(k == KT - 1),
                    )
            return ps1

        def emit_ps2_pair_and_g(c, ms, ps1, gs):
            _, nn = chunks[c]
            ps2 = {
                m: pspool.tile([P, nn], f32, tag="ps", name=f"ps2_{c}_{m}")
                for m in ms
            }
            for k in range(KT):
                for m in ms:
                    nc.tensor.matmul(
                        ps2[m][:], w2s(k, m), xs(c, k),
                        start=(k == 0), stop=(k == KT - 1),
                    )
            for m in ms:
                sil = spool.tile([P, nn], f32, tag="sil", name="sil")
                nc.scalar.activation(sil[:], ps1[m][:], silu)
                g = gpool.tile([P, nn], in_dt, tag=f"g{m}", name=f"g{m}")
                nc.vector.tensor_mul(g[:], sil[:], ps2[m][:])
                gs.append(g)

        def emit_stage_a0(gs):
            # chunk 0: fused W1+W2 k-loop over all 4 f-tiles so the PE
            # consumes one (w1,x0,w2) k-pair DMA triplet per 16 matmuls,
            # staying just behind the HWDGE delivery rate during preload
            nn = chunks[0][1]
            ps1 = {
                m: pspool.tile([P, nn], f32, tag="ps", name=f"ps1_0_{m}")
                for m in range(MT)
            }
            ps2 = {
                m: pspool.tile([P, nn], f32, tag="ps", name=f"ps2_0_{m}")
                for m in range(MT)
            }
            for k in range(KT):
                for m in range(MT):
                    nc.tensor.matmul(
                        ps1[m][:], w1s(k, m), xs(0, k),
                        start=(k == 0), stop=(k == KT - 1),
                    )
                for m in range(MT):
                    nc.tensor.matmul(
                        ps2[m][:], w2s(k, m), xs(0, k),
                        start=(k == 0), stop=(k == KT - 1),
                    )
            for m in range(MT):
                sil = spool.tile([P, nn], f32, tag="sil", name="sil")
                nc.scalar.activation(sil[:], ps1[m][:], silu)
                g = gpool.tile([P, nn], in_dt, tag=f"g{m}", name=f"g{m}")
                nc.vector.tensor_mul(g[:], sil[:], ps2[m][:])
                gs.append(g)

        def emit_stage_b(c, gs):
            n0, nn = chunks[c]
            last = c == len(chunks) - 1
            ot = opool.tile([P, DT, nn], f32, tag="ot", name="ot")
            # tail d-pair stores fan across SP/Act/Pool so the final chain
            # isn't serialized on one SEQ or on the shared HWDGE device
            # (Pool goes via SWDGE, bypassing HWDGE)
            if last:
                # tail chunk: an m0 round across all d first bridges the
                # trailing g2/g3 silu->mul latency; stores grouped so the
                # very last store (gated by d5's copy) is the smallest
                # possible transfer on its own free queue (Act), with d0-2
                # on SP/HWDGE and d3-4 on Pool/SWDGE ahead of it
                psos = [
                    pspool.tile([P, nn], f32, tag="ps", name=f"pso{d}")
                    for d in range(DT)
                ]
                for d in range(DT):
                    nc.tensor.matmul(
                        psos[d][:], w3s(0, d), gs[0][:], start=True, stop=False
                    )
                for d in range(DT):
                    for m in range(1, MT):
                        nc.tensor.matmul(
                            psos[d][:], w3s(m, d), gs[m][:],
                            start=False, stop=(m == MT - 1),
                        )
                    if d % 2 == 1:
                        nc.scalar.copy(ot[:, d, :], psos[d][:])
                    else:
                        nc.vector.tensor_copy(ot[:, d, :], psos[d][:])
                    if d % 2 == 1:
                        tail_eng = [nc.gpsimd, nc.sync, nc.scalar]
                        tail_eng[d // 2].dma_start(
                            yt[d - 1 : d + 1, :, n0 : n0 + nn].rearrange(
                                "d p n -> p d n"
                            ),
                            ot[:, d - 1 : d + 1, :],
                        )
                return
            for d in range(DT):
                pso = pspool.tile([P, nn], f32, tag="ps", name="pso")
                for m in range(MT):
                    nc.tensor.matmul(
                        pso[:], w3s(m, d), gs[m][:],
                        start=(m == 0), stop=(m == MT - 1),
                    )
                if d % 2 == 1:
                    nc.scalar.copy(ot[:, d, :], pso[:])
                else:
                    nc.vector.tensor_copy(ot[:, d, :], pso[:])
                if d % 2 == 1:
                    # store each d-pair as soon as both copies land. Thirds
                    # (not one merged store) so no single transfer hogs the
                    # serial DMA_ENGINES device into the next chunk's window.
                    # All on SP: idle after preload, and wait order matches
                    # time order there -- stores must NOT sit on Act/DVE
                    # queues where in-order SEQs would block compute
                    # dispatch behind the store's copy-waits.
                    nc.sync.dma_start(
                        yt[d - 1 : d + 1, :, n0 : n0 + nn].rearrange(
                            "d p n -> p d n"
                        ),
                        ot[:, d - 1 : d + 1, :],
                    )

        NCH = len(chunks)
        gs_all = [[] for _ in range(NCH)]
        hoisted = [None] * NCH
        emit_stage_a0(gs_all[0])
        for c in range(NCH):
            if c + 1 < NCH:
                # hoist: next chunk's ps1 pair-0 k-loop before this stage B
                hoisted[c + 1] = emit_ps1_pair(c + 1, (0, 1))
            emit_stage_b(c, gs_all[c])
            if c + 1 < NCH:
                emit_ps2_pair_and_g(c + 1, (0, 1), hoisted[c + 1], gs_all[c + 1])
                ps1b = emit_ps1_pair(c + 1, (2, 3))
                emit_ps2_pair_and_g(c + 1, (2, 3), ps1b, gs_all[c + 1])

    nc.compile()
    return nc


LAST_RESULTS = None  # BassKernelResults of the most recent run (for test harness)


def kernel(x, Wg, bg, W1, W2, W3):
    global LAST_RESULTS
    from concourse.bass_utils import run_bass_kernel_spmd

    x = np.asarray(x)
    Wg, bg = np.asarray(Wg), np.asarray(bg)
    W1, W2, W3 = np.asarray(W1), np.asarray(W2), np.asarray(W3)
    B, S, d = x.shape
    T = B * S
    assert d == D and Wg.shape == (E, D)

    xf = np.ascontiguousarray(x.reshape(T, D))

    # ---- host gate + top-1 routing (fp64: exact vs any fp32 backend) ----
    gate = xf.astype(np.float64) @ Wg.astype(np.float64).T + bg.astype(np.float64)
    eid = np.argmax(gate, axis=1)
    counts = np.bincount(eid, minlength=E)
    order = np.argsort(eid, kind="stable")
    offs = np.concatenate(([0], np.cumsum(counts)))

    C = max(MIN_C, 2 * int(-(-counts.max() // 2)))
    key = (C, MM_MODE)
    if key not in _cache:
        _cache[key] = _build(C)
    nc = _cache[key]

    in_dt = _np_in_dtype()

    # ---- build per-core inputs (dispatch) ----
    in_maps = []
    tok_lists = []
    for e in range(E):
        toks = order[offs[e] : offs[e + 1]]
        tok_lists.append(toks)
        ce = len(toks)
        xeT = np.zeros((D, C), dtype=in_dt)
        if ce:
            xeT[:, :ce] = xf[toks].T.astype(in_dt)
        w1 = np.zeros((D, FP), dtype=in_dt)
        w1[:, :F] = W1[e].T.astype(in_dt)
        w2 = np.zeros((D, FP), dtype=in_dt)
        w2[:, :F] = W2[e].T.astype(in_dt)
        w3 = np.zeros((FP, D), dtype=in_dt)
        w3[:F, :] = W3[e].T.astype(in_dt)
        in_maps.append(
            {
                "xt": np.ascontiguousarray(xeT.reshape(KT, P, C)),
                "w1t": np.ascontiguousarray(w1.reshape(KT, P, FP)),
                "w2t": np.ascontiguousarray(w2.reshape(KT, P, FP)),
                "w3t": np.ascontiguousarray(w3.reshape(MT, P, D)),
            }
        )

    res = run_bass_kernel_spmd(nc, in_maps, list(range(E)))
    LAST_RESULTS = res

    # ---- combine: scatter outputs back to token order ----
    y = np.empty((T, D), dtype=np.float32)
    for e in range(E):
        toks = tok_lists[e]
        if len(toks):
            yte = res.results[e]["yt"].reshape(D, C)
            y[toks] = yte[:, : len(toks)].T
    return y.reshape(B, S, d)
